# revision 7
# baseline (speedup 1.0000x reference)
"""Chamfer distance kernel for 8 TRN2 NeuronCores.

Problem: x, y of shape (8, 8192, 3) f32; output scalar
  sum_b max(mean_n min_m ||x_bn - y_bm||, mean_m min_n ||x_bn - y_bm||)

Sharding: batch-parallel, one batch element per core (B == n_cores == 8).
Each core computes its batch's scalar max(mean1, mean2); the host sums the
8 per-core scalars (the hint's single all-reduce, done at gather time).

Per-core algorithm (brute force, fused):
  The TensorEngine computes P[n, m] = x~.y~ - xx/2 - yy/2 = -dist^2/2 via a
  single K=16 matmul per tile: each f32 coordinate is split into an fp16
  hi/lo pair (x = xh + xl exactly to 2^-24), all four cross products
  (xh.yh, xh.yl, xl.yh, xl.yl) are K-rows, and the point norms (also split
  to fp16 pairs) ride along as extra K-rows against constant-one rows.
  fp16 matmuls stream at 1 col/cycle (vs 4 for fp32) and fp16 x fp16
  products accumulate exactly in fp32 PSUM, so this gives fp32-quality
  distances at bf16-rate.

  min_m dist^2 = -2 max_m P, and sqrt is monotonic, so each direction is a
  row-max over P tiles followed by one sqrt per point:
    distance1[n] = sqrt(-2 max_m P[n,m] + EPS)
  Direction 2 reuses the same two operand tensors with lhsT/rhs swapped.

  PSUM groups of [128, 2048] are drained by two engines in parallel:
  - DVE-direct groups: tensor_reduce(max) straight from PSUM (1x mode).
  - ACT groups: ScalarE casts PSUM f32 -> SBUF fp16, then DVE runs a 4x-mode
    fp16 tensor_scalar with a max accumulator (op0=min with 0.0 also clamps
    P <= 0 so dist^2 + EPS can't go negative into sqrt).
  The 4/11 direct fraction balances DVE vs ACT occupancy.
"""

import numpy as np
from contextlib import ExitStack

B = 8
NPOINTS = 8192
EPS = 1e-10
GROUP_FD = 2048
CHUNK = 512
# groups with (gidx % DIRECT_MOD) < DIRECT_CNT are reduced straight from
# PSUM on DVE; the rest go through the ACT fp16-cast path
DIRECT_MOD = 11
DIRECT_CNT = 4


def emit(tc, out_ap, x_ap, y_ap, n=NPOINTS, direct_mod=DIRECT_MOD,
         direct_cnt=DIRECT_CNT):
    """Emit the per-core chamfer kernel into TileContext tc.

    x_ap, y_ap: DRAM [n, 3] f32.  out_ap: DRAM [1, 1] f32.
    """
    import concourse.mybir as mybir
    from concourse.mybir import AluOpType as alu

    nc = tc.nc
    f32 = mybir.dt.float32
    f16 = mybir.dt.float16
    X = mybir.AxisListType.X
    ntile = n // 128
    groups = n // GROUP_FD

    ctx = ExitStack()
    with ctx:
        singles = ctx.enter_context(tc.tile_pool(name="singles", bufs=1))
        work = ctx.enter_context(tc.tile_pool(name="work", bufs=1))
        h16p = ctx.enter_context(tc.tile_pool(name="h16p", bufs=3))
        colp = ctx.enter_context(tc.tile_pool(name="colp", bufs=2))
        psum = ctx.enter_context(tc.tile_pool(name="psum", bufs=2, space="PSUM"))

        # Wide matmul operands. XW is "lhs-style": rows
        #   [xh0 xh1 xh2  xh0 xh1 xh2  xl0 xl1 xl2  xl0 xl1 xl2  1 1  nxh nxl]
        # YW is "rhs-style": rows
        #   [yh0 yh1 yh2  yl0 yl1 yl2  yh0 yh1 yh2  yl0 yl1 yl2  nyh nyl  1 1]
        # where nh/nl is the fp16 hi/lo split of -||p||^2/2.  Row k of the
        # lhsT always multiplies row k of the rhs, and both (XW lhsT, YW rhs)
        # and (YW lhsT, XW rhs) produce all four hi/lo cross products plus
        # the two norm terms.
        XW = singles.tile([16, n], f16, tag="XW")
        YW = singles.tile([16, n], f16, tag="YW")
        D1 = singles.tile([128, ntile], f32, tag="D1")
        D2 = singles.tile([128, ntile], f32, tag="D2")
        junk16 = singles.tile([128, GROUP_FD], f16, tag="junk16")
        junkg = singles.tile([128, groups], f16, tag="junkg")
        eps_col = singles.tile([128, 1], f32, tag="eps_col")
        nc.vector.memset(eps_col, EPS)
        ones2 = singles.tile([2, n], f16, tag="ones2")
        nc.vector.memset(ones2, 1.0)

        def prep(inp, W, lhs_style):
            # load t-major [128, 3t+d] = x[128t+p, d]
            Xw = work.tile([128, 3 * ntile], f32, tag="Xw")
            nc.sync.dma_start(
                out=Xw[:, :].rearrange("p (t d) -> p t d", d=3),
                in_=inp.rearrange("(t p) d -> p t d", p=128),
            )
            # d-major f32, padded to 4 components (cols d*ntile + t)
            Xd = work.tile([128, 4 * ntile], f32, tag="Xd")
            nc.vector.memset(Xd[:, 3 * ntile:], 0.0)
            nc.vector.tensor_copy(
                Xd[:, 0:3 * ntile].rearrange("p (d t) -> p d t", d=3),
                Xw[:, :].rearrange("p (t d) -> p d t", d=3),
            )
            # fp16 hi/lo split
            Xh = work.tile([128, 4 * ntile], f16, tag="Xh")
            nc.scalar.copy(Xh, Xd)
            Xl = work.tile([128, 4 * ntile], f16, tag="Xl")
            nc.vector.tensor_tensor(Xl, Xd, Xh, alu.subtract)
            # norms: -||p||^2/2 in t-major [128, ntile], then fp16 hi/lo
            Sq = work.tile([128, 3 * ntile], f32, tag="Sq")
            nc.scalar.square(Sq, Xw)
            sq3 = Sq[:, :].rearrange("p (t d) -> p d t", d=3)
            nxx = work.tile([128, ntile], f32, tag="nxx")
            nc.vector.tensor_tensor(nxx, sq3[:, 0, :], sq3[:, 1, :], alu.add)
            nc.vector.tensor_tensor(nxx, nxx, sq3[:, 2, :], alu.add)
            nc.vector.tensor_scalar_mul(nxx, nxx, -0.5)
            nrm = work.tile([128, 2 * ntile], f16, tag="nrm")
            nc.scalar.copy(nrm[:, 0:ntile], nxx)
            nc.vector.tensor_tensor(nrm[:, ntile:], nxx, nrm[:, 0:ntile],
                                    alu.subtract)

            # xbar transposes to (t, p)-major rows; free dim must be a
            # multiple of 128, partition dim of source is 128.
            def xp(src, cols, tag):
                t_ = work.tile([cols, 128], f16, tag=tag)
                nc.sync.dma_start_transpose(t_, src)
                return t_
            TA = xp(Xh[:, 0:2 * ntile], 2 * ntile, "TA")        # xh0, xh1
            TB = xp(Xh[:, 2 * ntile:4 * ntile], 2 * ntile, "TB")  # xh2, 0
            TC = xp(Xl[:, 0:2 * ntile], 2 * ntile, "TC")
            TD = xp(Xl[:, 2 * ntile:4 * ntile], 2 * ntile, "TD")
            TN = xp(nrm, 2 * ntile, "TN")                      # nxh, nxl

            h0, h1, h2 = (TA, 0), (TA, ntile), (TB, 0)
            l0, l1, l2 = (TC, 0), (TC, ntile), (TD, 0)
            nh, nl = (TN, 0), (TN, ntile)
            ONE = None
            if lhs_style:
                rows = [h0, h1, h2, h0, h1, h2, l0, l1, l2, l0, l1, l2,
                        ONE, ONE, nh, nl]
            else:
                rows = [h0, h1, h2, l0, l1, l2, h0, h1, h2, l0, l1, l2,
                        nh, nl, ONE, ONE]
            for r, src in enumerate(rows):
                if src is ONE:
                    continue
                T, off = src
                nc.sync.dma_start(out=W[r:r + 1, :], in_=T[off:off + ntile, :])
            one_base = 12 if lhs_style else 14
            nc.sync.dma_start(out=W[one_base:one_base + 2, :], in_=ones2[:, :])

        prep(x_ap, XW, True)
        prep(y_ap, YW, False)

        gidx = 0

        def direction(lhsW, rhsW, Dcols):
            nonlocal gidx
            for t in range(ntile):
                gcols = colp.tile([128, groups], f16, tag="gcols")
                for g in range(groups):
                    ps = psum.tile([128, GROUP_FD], f32, tag="ps")
                    for c in range(GROUP_FD // CHUNK):
                        m0 = g * GROUP_FD + c * CHUNK
                        nc.tensor.matmul(
                            ps[:, c * CHUNK:(c + 1) * CHUNK],
                            lhsW[:, t * 128:(t + 1) * 128],
                            rhsW[:, m0:m0 + CHUNK],
                            start=True, stop=True,
                        )
                    if (gidx % direct_mod) < direct_cnt:
                        nc.vector.tensor_reduce(gcols[:, g:g + 1], ps[:, :],
                                                axis=X, op=alu.max)
                    else:
                        h16 = h16p.tile([128, GROUP_FD], f16, tag="h16")
                        nc.scalar.copy(h16, ps)
                        nc.vector.tensor_scalar(junk16, h16, 0.0, None,
                                                alu.min, alu.max,
                                                accum_out=gcols[:, g:g + 1])
                    gidx += 1
                # clamp to <= 0 and reduce the per-group maxima
                pmax = colp.tile([128, 1], f16, tag="pmax")
                nc.vector.tensor_scalar(junkg, gcols, 0.0, None,
                                        alu.min, alu.max, accum_out=pmax)
                # distance = sqrt(-2 * max P + EPS)
                nc.scalar.activation(Dcols[:, t:t + 1], pmax,
                                     mybir.ActivationFunctionType.Sqrt,
                                     bias=eps_col[:, :], scale=-2.0)

        direction(XW, YW, D1)
        direction(YW, XW, D2)

        # mean over points, max of the two directions
        sums = singles.tile([128, 2], f32, tag="sums")
        nc.vector.tensor_reduce(sums[:, 0:1], D1[:, :], axis=X, op=alu.add)
        nc.vector.tensor_reduce(sums[:, 1:2], D2[:, :], axis=X, op=alu.add)
        ones = singles.tile([128, 1], f32, tag="ones")
        nc.vector.memset(ones, 1.0)
        pstail = psum.tile([128, GROUP_FD], f32, tag="ps")
        pq = pstail[0:1, 0:2]
        nc.tensor.matmul(pq, ones[:, :], sums[:, :], start=True, stop=True)
        fin = singles.tile([1, 2], f32, tag="fin")
        res = singles.tile([1, 1], f32, tag="res")
        nc.vector.tensor_scalar(fin, pq, 1.0 / n, None, alu.mult, alu.max,
                                accum_out=res)
        nc.sync.dma_start(out=out_ap, in_=res)


_NC_CACHE = {}


def build(n=NPOINTS, direct_mod=DIRECT_MOD, direct_cnt=DIRECT_CNT):
    key = (n, direct_mod, direct_cnt)
    if key in _NC_CACHE:
        return _NC_CACHE[key]
    import concourse.mybir as mybir
    import concourse.tile as tile
    from concourse import bacc

    nc = bacc.Bacc(None, target_bir_lowering=False)
    x = nc.dram_tensor("x", [n, 3], mybir.dt.float32, kind="ExternalInput")
    y = nc.dram_tensor("y", [n, 3], mybir.dt.float32, kind="ExternalInput")
    out = nc.dram_tensor("out", [1, 1], mybir.dt.float32, kind="ExternalOutput")
    with tile.TileContext(nc) as tc:
        emit(tc, out[:, :], x[:, :], y[:, :], n=n,
             direct_mod=direct_mod, direct_cnt=direct_cnt)
    nc.finalize()
    _NC_CACHE[key] = nc
    return nc


def kernel(x, y):
    """Full-input entry point: x, y (8, 8192, 3) f32 -> scalar f32."""
    from concourse.bass_utils import run_bass_kernel_spmd

    x = np.asarray(x, dtype=np.float32)
    y = np.asarray(y, dtype=np.float32)
    assert x.shape == (B, NPOINTS, 3) and y.shape == (B, NPOINTS, 3)
    nc = build()
    in_maps = [
        {"x": np.ascontiguousarray(x[b]), "y": np.ascontiguousarray(y[b])}
        for b in range(B)
    ]
    res = run_bass_kernel_spmd(nc, in_maps, core_ids=list(range(B)))
    total = np.float32(0.0)
    for r in res.results:
        total = np.float32(total + np.float32(r["out"][0, 0]))
    return total


# revision 23
# speedup vs baseline: 3.5305x; 3.5305x over previous
"""Chamfer distance kernel for 8 TRN2 NeuronCores.

Problem: x, y of shape (8, 8192, 3) f32; output scalar
  sum_b max(mean_n min_m ||x_bn - y_bm||, mean_m min_n ||x_bn - y_bm||)

Sharding: batch-parallel, one batch element per core (B == n_cores == 8).
Each core computes its batch's scalar max(mean1, mean2); the host sums the
8 per-core scalars (the hint's single all-reduce, done at gather time).

Per-core algorithm (brute force, fused):
  The TensorEngine computes P[n, m] = x~.y~ - xx/2 - yy/2 = -dist^2/2 via a
  single K=16 matmul per tile: each f32 coordinate is split into an fp16
  hi/lo pair (x = xh + xl exactly to 2^-24), all four cross products
  (xh.yh, xh.yl, xl.yh, xl.yl) are K-rows, and the point norms (also split
  to fp16 pairs) ride along as extra K-rows against constant-one rows.
  fp16 matmuls stream at 1 col/cycle (vs 4 for fp32) and fp16 x fp16
  products accumulate exactly in fp32 PSUM, so this gives fp32-quality
  distances at bf16-rate.

  min_m dist^2 = -2 max_m P, and sqrt is monotonic, so each direction is a
  row-max over P tiles followed by one sqrt per point:
    distance1[n] = sqrt(-2 max_m P[n,m] + EPS)
  Direction 2 reuses the same two operand tensors with lhsT/rhs swapped.

  PSUM groups of [128, 2048] (double-buffered, filling all 8 banks) are
  drained by DVE tensor_reduce(max) straight from PSUM. Measured on HW,
  PSUM reads barely overlap PE writes or a second reader (ACT), so the
  mixed DVE+ACT drain schedules and serial write/read phasings all lose
  to this simple form. The per-group maxima are clamped to <= 0 (P <= 0
  exactly; clamping guards sqrt against representation noise on
  near-duplicate points) and reduced per n-tile, then ScalarE applies
  sqrt(-2*max + EPS).
"""

import numpy as np
from contextlib import ExitStack

B = 8
NPOINTS = 8192
EPS = 1e-10
GROUP_FD = 2048
CHUNK = 512
# groups with (gidx % DIRECT_MOD) < DIRECT_CNT are reduced straight from
# PSUM on DVE; the rest go through the ACT fp16-cast path. 1/1 = all direct:
# on this silicon the ACT-assisted path never beat pure DVE tensor_reduce
# (PSUM reads barely overlap across engines), so the default is all-direct.
DIRECT_MOD = 1
DIRECT_CNT = 1


def emit(tc, out_ap, x_ap, y_ap, n=NPOINTS, direct_mod=DIRECT_MOD,
         direct_cnt=DIRECT_CNT, reps=1, ablate=None, group_fd=GROUP_FD,
         psum_bufs=2, h16_bufs=3, tiled=True):
    """Emit the per-core chamfer kernel into TileContext tc.

    x_ap, y_ap: DRAM [n, 3] f32.  out_ap: DRAM [1, 1] f32.
    """
    import concourse.mybir as mybir
    from concourse.mybir import AluOpType as alu

    nc = tc.nc
    f32 = mybir.dt.float32
    f16 = mybir.dt.float16
    X = mybir.AxisListType.X
    ntile = n // 128
    groups = n // group_fd

    ctx = ExitStack()
    with ctx:
        singles = ctx.enter_context(tc.tile_pool(name="singles", bufs=1))
        work = ctx.enter_context(tc.tile_pool(name="work", bufs=1))
        h16p = ctx.enter_context(tc.tile_pool(name="h16p", bufs=h16_bufs))
        colp = ctx.enter_context(tc.tile_pool(name="colp", bufs=2))
        if ablate == "mono":
            psum_bufs = 1
        psum = ctx.enter_context(tc.tile_pool(name="psum", bufs=psum_bufs, space="PSUM"))

        # Wide matmul operands. XW is "lhs-style": rows
        #   [xh0 xh1 xh2  xh0 xh1 xh2  xl0 xl1 xl2  xl0 xl1 xl2  1 1  nxh nxl]
        # YW is "rhs-style": rows
        #   [yh0 yh1 yh2  yl0 yl1 yl2  yh0 yh1 yh2  yl0 yl1 yl2  nyh nyl  1 1]
        # where nh/nl is the fp16 hi/lo split of -||p||^2/2.  Row k of the
        # lhsT always multiplies row k of the rhs, and both (XW lhsT, YW rhs)
        # and (YW lhsT, XW rhs) produce all four hi/lo cross products plus
        # the two norm terms.
        # The 16 rows are replicated at partition bases 0/32/64/96 so the PE
        # can run in 32-row-tiled mode: 4 independent 32x128 tiles with
        # parallel weight loads + streams (~3.7x faster than one 128x128).
        XW = singles.tile([128, n], f16, tag="XW")
        YW = singles.tile([128, n], f16, tag="YW")
        D1 = singles.tile([128, ntile], f32, tag="D1")
        D2 = singles.tile([128, ntile], f32, tag="D2")
        junk16 = singles.tile([128, group_fd], f16, tag="junk16")
        junkg = singles.tile([128, max(16, groups)], f16, tag="junkg")
        eps_col = singles.tile([128, 1], f32, tag="eps_col")
        nc.vector.memset(eps_col, EPS)
        ones2 = singles.tile([2, n], f16, tag="ones2")
        nc.vector.memset(ones2, 1.0)

        def prep(inp, W, lhs_style):
            # load t-major [128, 3t+d] = x[128t+p, d]
            Xw = work.tile([128, 3 * ntile], f32, tag="Xw")
            nc.sync.dma_start(
                out=Xw[:, :].rearrange("p (t d) -> p t d", d=3),
                in_=inp.rearrange("(t p) d -> p t d", p=128),
            )
            # d-major f32, padded to 4 components (cols d*ntile + t)
            Xd = work.tile([128, 4 * ntile], f32, tag="Xd")
            nc.vector.memset(Xd[:, 3 * ntile:], 0.0)
            nc.vector.tensor_copy(
                Xd[:, 0:3 * ntile].rearrange("p (d t) -> p d t", d=3),
                Xw[:, :].rearrange("p (t d) -> p d t", d=3),
            )
            # fp16 hi/lo split
            Xh = work.tile([128, 4 * ntile], f16, tag="Xh")
            nc.scalar.copy(Xh, Xd)
            Xl = work.tile([128, 4 * ntile], f16, tag="Xl")
            nc.vector.tensor_tensor(Xl, Xd, Xh, alu.subtract)
            # norms: -||p||^2/2 in t-major [128, ntile], then fp16 hi/lo
            Sq = work.tile([128, 3 * ntile], f32, tag="Sq")
            nc.scalar.square(Sq, Xw)
            sq3 = Sq[:, :].rearrange("p (t d) -> p d t", d=3)
            nxx = work.tile([128, ntile], f32, tag="nxx")
            nc.vector.tensor_tensor(nxx, sq3[:, 0, :], sq3[:, 1, :], alu.add)
            nc.vector.tensor_tensor(nxx, nxx, sq3[:, 2, :], alu.add)
            nc.vector.tensor_scalar_mul(nxx, nxx, -0.5)
            nrm = work.tile([128, 2 * ntile], f16, tag="nrm")
            nc.scalar.copy(nrm[:, 0:ntile], nxx)
            nc.vector.tensor_tensor(nrm[:, ntile:], nxx, nrm[:, 0:ntile],
                                    alu.subtract)

            # xbar transposes to (t, p)-major rows; free dim must be a
            # multiple of 128, partition dim of source is 128.
            def xp(src, cols, tag):
                t_ = work.tile([cols, 128], f16, tag=tag)
                nc.sync.dma_start_transpose(t_, src)
                return t_
            TA = xp(Xh[:, 0:2 * ntile], 2 * ntile, "TA")        # xh0, xh1
            TB = xp(Xh[:, 2 * ntile:4 * ntile], 2 * ntile, "TB")  # xh2, 0
            TC = xp(Xl[:, 0:2 * ntile], 2 * ntile, "TC")
            TD = xp(Xl[:, 2 * ntile:4 * ntile], 2 * ntile, "TD")
            TN = xp(nrm, 2 * ntile, "TN")                      # nxh, nxl

            h0, h1, h2 = (TA, 0), (TA, ntile), (TB, 0)
            l0, l1, l2 = (TC, 0), (TC, ntile), (TD, 0)
            nh, nl = (TN, 0), (TN, ntile)
            ONE = None
            if lhs_style:
                rows = [h0, h1, h2, h0, h1, h2, l0, l1, l2, l0, l1, l2,
                        ONE, ONE, nh, nl]
            else:
                rows = [h0, h1, h2, l0, l1, l2, h0, h1, h2, l0, l1, l2,
                        nh, nl, ONE, ONE]
            for r, src in enumerate(rows):
                if src is ONE:
                    continue
                T, off = src
                nc.sync.dma_start(out=W[r:r + 1, :], in_=T[off:off + ntile, :])
            one_base = 12 if lhs_style else 14
            nc.sync.dma_start(out=W[one_base:one_base + 2, :], in_=ones2[:, :])
            # replicate rows 0-15 into the other three PE-array quadrants
            for q in (32, 64, 96):
                nc.sync.dma_start(out=W[q:q + 16, :], in_=W[0:16, :])

        prep(x_ap, XW, True)
        prep(y_ap, YW, False)
        if ablate in ("mmonly", "nomm"):
            nc.vector.memset(D1[:, :], 0.0)
            nc.vector.memset(D2[:, :], 0.0)

        gidx = 0

        def direction_mono(lhsW, rhsW, Dcols):
            # serial write/read phasing: fill all 8 PSUM banks with 8 tiled
            # matmuls, then one whole-PSUM [128, 4096] reduce. Avoids the
            # PSUM read-under-write bandwidth collapse.
            nphase = n // 4096
            for t in range(ntile):
                gcols = colp.tile([128, nphase], f16, tag="gcols")
                for h in range(nphase):
                    ps = psum.tile([128, 4096], f32, tag="ps")
                    for c in range(8):
                        m0 = h * 4096 + c * CHUNK
                        q = 32 * (c % 4)
                        nc.tensor.matmul(
                            ps[:, c * CHUNK:(c + 1) * CHUNK],
                            lhsW[q:q + 16, t * 128:(t + 1) * 128],
                            rhsW[q:q + 16, m0:m0 + CHUNK],
                            start=True, stop=True,
                            tile_position=(q, 0),
                        )
                    nc.vector.tensor_reduce(gcols[:, h:h + 1], ps[:, :],
                                            axis=X, op=alu.max)
                pmax = colp.tile([128, 1], f16, tag="pmax")
                nc.vector.tensor_scalar(junkg[:, 0:nphase], gcols, 0.0, None,
                                        alu.min, alu.max, accum_out=pmax)
                nc.scalar.activation(Dcols[:, t:t + 1], pmax,
                                     mybir.ActivationFunctionType.Sqrt,
                                     bias=eps_col[:, :], scale=-2.0)

        def direction(lhsW, rhsW, Dcols):
            nonlocal gidx
            gw = groups * (2 if ablate == "split2" else 4 if ablate == "split4" else 1)
            for t in range(ntile):
                gcols = colp.tile([128, gw], f16, tag="gcols")
                for g in range(groups):
                    ps = psum.tile([128, group_fd], f32, tag="ps")
                    if ablate != "nomm":
                        for c in range(group_fd // CHUNK):
                            m0 = g * group_fd + c * CHUNK
                            q = 32 * (c % 4) if tiled else 0
                            nc.tensor.matmul(
                                ps[:, c * CHUNK:(c + 1) * CHUNK],
                                lhsW[q:q + 16, t * 128:(t + 1) * 128],
                                rhsW[q:q + 16, m0:m0 + CHUNK],
                                start=True, stop=True,
                                tile_position=(q, 0) if tiled else None,
                            )
                    if ablate == "mmonly" or ablate == "nomm":
                        gidx += 1
                        continue
                    if ablate in ("split2", "split4"):
                        nsp = 2 if ablate == "split2" else 4
                        w_ = group_fd // nsp
                        for s_ in range(nsp):
                            nc.vector.tensor_reduce(
                                gcols[:, g * nsp + s_:g * nsp + s_ + 1],
                                ps[:, s_ * w_:(s_ + 1) * w_], axis=X, op=alu.max)
                    elif ablate == "alldirect" or (
                            ablate is None and (gidx % direct_mod) < direct_cnt):
                        nc.vector.tensor_reduce(gcols[:, g:g + 1], ps[:, :],
                                                axis=X, op=alu.max)
                    elif ablate == "allact" or ablate is None:
                        h16 = h16p.tile([128, group_fd], f16, tag="h16")
                        nc.scalar.copy(h16, ps)
                        nc.vector.tensor_scalar(junk16, h16, 0.0, None,
                                                alu.min, alu.max,
                                                accum_out=gcols[:, g:g + 1])
                    gidx += 1
                if ablate in ("mmonly", "nomm"):
                    continue
                # clamp to <= 0 and reduce the per-group maxima
                pmax = colp.tile([128, 1], f16, tag="pmax")
                nc.vector.tensor_scalar(junkg[:, 0:gw], gcols, 0.0, None,
                                        alu.min, alu.max, accum_out=pmax)
                # distance = sqrt(-2 * max P + EPS)
                nc.scalar.activation(Dcols[:, t:t + 1], pmax,
                                     mybir.ActivationFunctionType.Sqrt,
                                     bias=eps_col[:, :], scale=-2.0)

        for _rep in range(reps):
            if ablate == "mono":
                direction_mono(XW, YW, D1)
                direction_mono(YW, XW, D2)
            else:
                direction(XW, YW, D1)
                direction(YW, XW, D2)

        # mean over points, max of the two directions
        sums = singles.tile([128, 2], f32, tag="sums")
        nc.vector.tensor_reduce(sums[:, 0:1], D1[:, :], axis=X, op=alu.add)
        nc.vector.tensor_reduce(sums[:, 1:2], D2[:, :], axis=X, op=alu.add)
        ones = singles.tile([128, 1], f32, tag="ones")
        nc.vector.memset(ones, 1.0)
        pstail = psum.tile([128, group_fd], f32, tag="ps")
        pq = pstail[0:1, 0:2]
        nc.tensor.matmul(pq, ones[:, :], sums[:, :], start=True, stop=True)
        fin = singles.tile([1, 2], f32, tag="fin")
        res = singles.tile([1, 1], f32, tag="res")
        nc.vector.tensor_scalar(fin, pq, 1.0 / n, None, alu.mult, alu.max,
                                accum_out=res)
        nc.sync.dma_start(out=out_ap, in_=res)


_NC_CACHE = {}


def build(n=NPOINTS, direct_mod=DIRECT_MOD, direct_cnt=DIRECT_CNT, reps=1,
          ablate=None, group_fd=GROUP_FD, psum_bufs=2, h16_bufs=3, tiled=True):
    key = (n, direct_mod, direct_cnt, reps, ablate, group_fd, psum_bufs,
           h16_bufs, tiled)
    if key in _NC_CACHE:
        return _NC_CACHE[key]
    import concourse.mybir as mybir
    import concourse.tile as tile
    from concourse import bacc

    nc = bacc.Bacc(None, target_bir_lowering=False)
    x = nc.dram_tensor("x", [n, 3], mybir.dt.float32, kind="ExternalInput")
    y = nc.dram_tensor("y", [n, 3], mybir.dt.float32, kind="ExternalInput")
    out = nc.dram_tensor("out", [1, 1], mybir.dt.float32, kind="ExternalOutput")
    with tile.TileContext(nc) as tc:
        emit(tc, out[:, :], x[:, :], y[:, :], n=n,
             direct_mod=direct_mod, direct_cnt=direct_cnt, reps=reps,
             ablate=ablate, group_fd=group_fd, psum_bufs=psum_bufs,
             h16_bufs=h16_bufs, tiled=tiled)
    nc.finalize()
    _NC_CACHE[key] = nc
    return nc


def kernel(x, y):
    """Full-input entry point: x, y (8, 8192, 3) f32 -> scalar f32."""
    from concourse.bass_utils import run_bass_kernel_spmd

    x = np.asarray(x, dtype=np.float32)
    y = np.asarray(y, dtype=np.float32)
    assert x.shape == (B, NPOINTS, 3) and y.shape == (B, NPOINTS, 3)
    nc = build()
    in_maps = [
        {"x": np.ascontiguousarray(x[b]), "y": np.ascontiguousarray(y[b])}
        for b in range(B)
    ]
    res = run_bass_kernel_spmd(nc, in_maps, core_ids=list(range(B)))
    total = np.float32(0.0)
    for r in res.results:
        total = np.float32(total + np.float32(r["out"][0, 0]))
    return total
